# revision 1
# baseline (speedup 1.0000x reference)
"""KGE (TransR-style) loss kernel for Trainium2, 8 NeuronCores.

Strategy:
  - Host: sort the M=8192 triples by relation id (pure index manipulation),
    pad each relation's segment to 128-row blocks -> ~96 single-relation
    blocks, distributed evenly across the 8 cores (same block count per
    core, so one SPMD program serves all cores). Per-core relation tables
    (W blocks, r rows) are sharded host-side per the block list.
  - Device (per core, per block b):
      * three indirect DMAs gather the h/pos/neg entity rows into
        X = [H | P | N]  (128 x 384)   [GPSIMD/SWDGE]
      * D_pos = H - P, D_neg = H - N; squares + row reductions  [DVE]
      * PE transpose D -> D^T; ACT copies PSUM->SBUF
      * matmul D^T.T @ W_b accumulated with a K=NB one-hot matmul adding
        r_b -> (h - t) @ W + r in PSUM  [PE]
      * score diff col stored per block; softplus tail batched over all
        blocks at the end (2 act-table loads total instead of ~2/block)
  - reg = 0.5*sum(X^2) per row, masked+scaled by 1e-5 via the wval input;
    relation-embedding reg via per-block counts.
  - Final: free-dim reduce + ones-matmul partition reduce -> one f32 per
    core; host sums the 8 partials and divides by M.
"""

import os
from contextlib import ExitStack

import numpy as np

import concourse.bass as bass
import concourse.tile as tile
from concourse import bacc, mybir
from concourse.masks import make_identity

M = 8192
E = 128
N_ENT = 500000
N_REL = 64
LAM = 1e-5
P = 128
N_CORES = 8
PAD_BIAS = -30000.0

f32 = mybir.dt.float32
i32 = mybir.dt.int32

_cache = {}


def _build(NB: int):
    """Build + compile the single-core SPMD program for NB blocks/core."""
    nc = bacc.Bacc(
        "TRN2",
        target_bir_lowering=False,
        debug=False,
        num_devices=N_CORES,
    )

    ent = nc.dram_tensor("ent", (N_ENT, E), f32, kind="ExternalInput").ap()
    idx3 = nc.dram_tensor("idx3", (P, NB * 3), i32, kind="ExternalInput").ap()
    mbias = nc.dram_tensor("mbias", (P, NB), f32, kind="ExternalInput").ap()
    wval = nc.dram_tensor("wval", (P, NB), f32, kind="ExternalInput").ap()
    w_all = nc.dram_tensor("w_all", (P, NB * P), f32, kind="ExternalInput").ap()
    r_in = nc.dram_tensor("r_blk", (NB, E), f32, kind="ExternalInput").ap()
    lsel = nc.dram_tensor("lsel", (NB, NB * P), f32, kind="ExternalInput").ap()
    cnt = nc.dram_tensor("cnt", (NB, 1), f32, kind="ExternalInput").ap()
    out = nc.dram_tensor("out", (1, 1), f32, kind="ExternalOutput").ap()

    with tile.TileContext(nc) as tc, ExitStack() as ctx:
        const = ctx.enter_context(tc.tile_pool(name="const", bufs=1))
        xp = ctx.enter_context(tc.tile_pool(name="xp", bufs=6))
        dp = ctx.enter_context(tc.tile_pool(name="dp", bufs=3))
        dtp = ctx.enter_context(tc.tile_pool(name="dtp", bufs=3))
        scrp = ctx.enter_context(tc.tile_pool(name="scrp", bufs=3))
        colp = ctx.enter_context(tc.tile_pool(name="colp", bufs=4))
        ps_t = ctx.enter_context(tc.tile_pool(name="ps_t", bufs=2, space="PSUM"))
        ps_mm = ctx.enter_context(tc.tile_pool(name="ps_mm", bufs=2, space="PSUM"))

        # constants / small inputs
        iden = const.tile([P, P], f32)
        make_identity(nc, iden[:])
        ones_col = const.tile([P, 1], f32)
        nc.gpsimd.memset(ones_col[:], 1.0)

        idx3_sb = const.tile([P, NB * 3], i32)
        nc.sync.dma_start(out=idx3_sb[:], in_=idx3[:])
        mb_sb = const.tile([P, NB], f32)
        nc.sync.dma_start(out=mb_sb[:], in_=mbias[:])
        wv_sb = const.tile([P, NB], f32)
        nc.sync.dma_start(out=wv_sb[:], in_=wval[:])
        cnt_sb = const.tile([NB, 1], f32)
        nc.sync.dma_start(out=cnt_sb[:], in_=cnt[:])
        w_sb = const.tile([P, NB * P], f32)
        nc.sync.dma_start(out=w_sb[:], in_=w_all[:])
        r_blk = const.tile([NB, E], f32)
        nc.sync.dma_start(out=r_blk[:], in_=r_in[:])
        lsel_sb = const.tile([NB, NB * P], f32)
        nc.sync.dma_start(out=lsel_sb[:], in_=lsel[:])

        # per-block score-diff columns and raw reg columns
        dcols = const.tile([P, NB], f32)
        regs = const.tile([P, NB], f32)

        for b in range(NB):
            # three gathers: hardware indirect DMA takes one index per
            # partition and reads out.free_size contiguous elems from it
            x = xp.tile([P, 3 * E], f32, tag="x")
            for j in range(3):
                nc.gpsimd.indirect_dma_start(
                    out=x[:, j * E : (j + 1) * E],
                    out_offset=None,
                    in_=ent[:],
                    in_offset=bass.IndirectOffsetOnAxis(
                        ap=idx3_sb[:, 3 * b + j : 3 * b + j + 1], axis=0
                    ),
                )

            # raw reg col: sum over [H|P|N] of squares (mask+scale at tail);
            # ACT Square with accum_out frees the DVE for score work
            xsq = scrp.tile([P, 3 * E], f32, tag="xsq")
            nc.scalar.activation(
                out=xsq[:], in_=x[:],
                func=mybir.ActivationFunctionType.Square,
                accum_out=regs[:, b : b + 1],
            )

            # D_pos = H - P, D_neg = H - N
            d_pos = dp.tile([P, E], f32, tag="dpos")
            nc.vector.tensor_tensor(
                out=d_pos[:], in0=x[:, 0:E], in1=x[:, E : 2 * E],
                op=mybir.AluOpType.subtract,
            )
            d_neg = dp.tile([P, E], f32, tag="dneg")
            nc.vector.tensor_tensor(
                out=d_neg[:], in0=x[:, 0:E], in1=x[:, 2 * E : 3 * E],
                op=mybir.AluOpType.subtract,
            )

            # transpose D -> D^T (PSUM), copy to SBUF on ACT
            dpt_ps = ps_t.tile([P, P], f32, tag="tp")
            nc.tensor.transpose(out=dpt_ps[:], in_=d_pos[:], identity=iden[:])
            dnt_ps = ps_t.tile([P, P], f32, tag="tn")
            nc.tensor.transpose(out=dnt_ps[:], in_=d_neg[:], identity=iden[:])
            dpt = dtp.tile([P, P], f32, tag="dpt")
            nc.scalar.copy(dpt[:], dpt_ps[:])
            dnt = dtp.tile([P, P], f32, tag="dnt")
            nc.scalar.copy(dnt[:], dnt_ps[:])

            # (h - t) @ W + r
            wb = w_sb[:, b * P : (b + 1) * P]
            lb = lsel_sb[:, b * P : (b + 1) * P]
            pos_ps = ps_mm.tile([P, E], f32, tag="mp")
            nc.tensor.matmul(out=pos_ps[:], lhsT=dpt[:], rhs=wb, start=True, stop=False)
            nc.tensor.matmul(out=pos_ps[:], lhsT=lb, rhs=r_blk[:], start=False, stop=True)
            neg_ps = ps_mm.tile([P, E], f32, tag="mn")
            nc.tensor.matmul(out=neg_ps[:], lhsT=dnt[:], rhs=wb, start=True, stop=False)
            nc.tensor.matmul(out=neg_ps[:], lhsT=lb, rhs=r_blk[:], start=False, stop=True)

            # score diff col (x2): sum(neg^2) - sum(pos^2); ACT Square reads
            # PSUM (DVE cannot read two PSUM inputs) and fuses the reduction
            psq = scrp.tile([P, E], f32, tag="psq")
            spos = colp.tile([P, 1], f32, tag="sp")
            nc.scalar.activation(
                out=psq[:], in_=pos_ps[:],
                func=mybir.ActivationFunctionType.Square,
                accum_out=spos[:],
            )
            nsq = scrp.tile([P, E], f32, tag="nsq")
            sneg = colp.tile([P, 1], f32, tag="sn")
            nc.scalar.activation(
                out=nsq[:], in_=neg_ps[:],
                func=mybir.ActivationFunctionType.Square,
                accum_out=sneg[:],
            )
            nc.vector.tensor_tensor(
                out=dcols[:, b : b + 1], in0=sneg[:], in1=spos[:],
                op=mybir.AluOpType.subtract,
            )

        # ---- batched tail over all NB blocks ----
        # loss = softplus(0.5*dcols + mbias) = relu(y) + ln(1 + exp(-|y|))
        dm = const.tile([P, NB], f32)
        nc.vector.tensor_scalar_mul(out=dm[:], in0=dcols[:], scalar1=0.5)
        nc.vector.tensor_tensor(
            out=dm[:], in0=dm[:], in1=mb_sb[:], op=mybir.AluOpType.add
        )
        t_abs = const.tile([P, NB], f32)
        nc.scalar.activation(
            out=t_abs[:], in_=dm[:], func=mybir.ActivationFunctionType.Abs
        )
        t_exp = const.tile([P, NB], f32)
        nc.scalar.activation(
            out=t_exp[:], in_=t_abs[:], func=mybir.ActivationFunctionType.Exp,
            scale=-1.0,
        )
        t_ln = const.tile([P, NB], f32)
        nc.scalar.activation(
            out=t_ln[:], in_=t_exp[:], func=mybir.ActivationFunctionType.Ln,
            bias=1.0,
        )
        t_relu = const.tile([P, NB], f32)
        nc.scalar.activation(
            out=t_relu[:], in_=dm[:], func=mybir.ActivationFunctionType.Relu
        )

        acc = const.tile([P, 2 * NB], f32)
        nc.vector.tensor_tensor(
            out=acc[:, :NB], in0=t_ln[:], in1=t_relu[:], op=mybir.AluOpType.add
        )
        # reg masked + scaled (wval holds 0.5*1e-5 or 0)
        nc.vector.tensor_tensor(
            out=acc[:, NB:], in0=regs[:], in1=wv_sb[:], op=mybir.AluOpType.mult
        )

        # relation-embedding reg: cnt_b * 0.5*||r_b||^2 (cnt pre-scaled 1e-5)
        rsq = const.tile([NB, E], f32)
        nc.vector.tensor_tensor(
            out=rsq[:], in0=r_blk[:], in1=r_blk[:], op=mybir.AluOpType.mult
        )
        rr_col = const.tile([NB, 1], f32)
        nc.vector.reduce_sum(out=rr_col[:], in_=rsq[:], axis=mybir.AxisListType.X)
        rr_s = const.tile([NB, 1], f32)
        nc.vector.tensor_tensor(
            out=rr_s[:], in0=rr_col[:], in1=cnt_sb[:], op=mybir.AluOpType.mult
        )

        # total per-partition, then partition-reduce via ones matmul
        t_all = const.tile([P, 1], f32)
        nc.vector.reduce_sum(out=t_all[:], in_=acc[:], axis=mybir.AxisListType.X)
        nc.vector.tensor_tensor(
            out=t_all[:NB], in0=t_all[:NB], in1=rr_s[:], op=mybir.AluOpType.add
        )
        fin_ps = ps_mm.tile([1, 1], f32, tag="mp")
        nc.tensor.matmul(out=fin_ps[:], lhsT=t_all[:], rhs=ones_col[:], start=True, stop=True)
        fin_sb = const.tile([1, 1], f32)
        nc.scalar.copy(fin_sb[:], fin_ps[:])
        nc.sync.dma_start(out=out[:], in_=fin_sb[:])

    nc.compile()
    return nc


def _plan(h, r, pos_t, neg_t, relation_weight, relation_embed):
    """Sort by relation, pad to 128-row single-relation blocks, split 8 ways."""
    order = np.argsort(r, kind="stable")
    counts = np.bincount(r, minlength=N_REL)
    blocks = []
    pos = 0
    for k in range(N_REL):
        c = int(counts[k])
        ids = order[pos : pos + c]
        pos += c
        for s in range(0, c, P):
            blocks.append((k, ids[s : s + P]))
    nb = max(2, -(-len(blocks) // N_CORES))
    while len(blocks) < nb * N_CORES:
        blocks.append((0, np.empty(0, np.int64)))

    maps = []
    for c in range(N_CORES):
        core_blocks = blocks[c * nb : (c + 1) * nb]
        idx3 = np.zeros((P, nb, 3), np.int32)
        mb = np.full((P, nb), PAD_BIAS, np.float32)
        wv = np.zeros((P, nb), np.float32)
        cnt = np.zeros((nb, 1), np.float32)
        w_blk = np.zeros((P, nb, P), np.float32)
        r_blk = np.zeros((nb, E), np.float32)
        for b, (k, ids) in enumerate(core_blocks):
            n = len(ids)
            if n:
                idx3[:n, b, 0] = h[ids]
                idx3[:n, b, 1] = pos_t[ids]
                idx3[:n, b, 2] = neg_t[ids]
            mb[:n, b] = 0.0
            wv[:n, b] = 0.5 * LAM
            cnt[b, 0] = n * LAM
            w_blk[:, b, :] = relation_weight[k]
            r_blk[b, :] = relation_embed[k]
        maps.append(
            {
                "idx3": idx3.reshape(P, nb * 3),
                "mbias": mb,
                "wval": wv,
                "cnt": cnt,
                "w_all": np.ascontiguousarray(w_blk.reshape(P, nb * P)),
                "r_blk": r_blk,
                "lsel": np.kron(np.eye(nb, dtype=np.float32), np.ones((1, P), np.float32)),
            }
        )
    return nb, maps


def kernel(h, r, pos_t, neg_t, entity_embed, relation_embed, relation_weight):
    h = np.asarray(h).astype(np.int32)
    r = np.asarray(r).astype(np.int32)
    pos_t = np.asarray(pos_t).astype(np.int32)
    neg_t = np.asarray(neg_t).astype(np.int32)
    ent = np.ascontiguousarray(np.asarray(entity_embed, dtype=np.float32))
    re = np.ascontiguousarray(np.asarray(relation_embed, dtype=np.float32))
    rw = np.ascontiguousarray(np.asarray(relation_weight, dtype=np.float32))

    nb, maps = _plan(h, r, pos_t, neg_t, rw, re)
    if nb not in _cache:
        _cache[nb] = _build(nb)
    nc = _cache[nb]

    in_maps = [{"ent": ent, **maps[c]} for c in range(N_CORES)]

    if os.environ.get("KGE_SIM"):
        from concourse.bass_interp import CoreSim

        total = 0.0
        for c in range(N_CORES):
            sim = CoreSim(nc, trace=False)
            for name, arr in in_maps[c].items():
                sim.tensor(name)[:] = arr
            sim.simulate()
            total += float(sim.tensor("out")[0, 0])
        return np.float32(total / M)

    from concourse.bass_utils import run_bass_kernel_spmd

    res = run_bass_kernel_spmd(nc, in_maps, core_ids=list(range(N_CORES)))
    total = sum(float(res.results[c]["out"][0, 0]) for c in range(N_CORES))
    return np.float32(total / M)



# revision 8
# speedup vs baseline: 2.1927x; 2.1927x over previous
"""KGE (TransR-style) loss kernel for Trainium2, 8 NeuronCores.

Strategy (v2):
  - Host: choose a relation ORDER such that after sorting rows by relation,
    no 128-row tile contains more than one relation boundary (greedy over
    prefix sums; counts are ~Poisson(128) so this nearly always succeeds;
    falls back to random restarts). Rows split 1024/core -> 8 tiles/core,
    ZERO padding.
  - Mixed-relation tiles are handled with data (not structure): per tile j
    the program computes  vT = W_A^T dT + W_D^T (dT*mask) + [r_A; r_D]-add
    where W_D = W_B - W_A and mask is the suffix indicator of the boundary
    row. Tiles without a boundary get W_D = 0. This keeps ONE fixed SPMD
    program for all cores/collations.
  - Device (per core):
      * 4 batched indirect DMAs gather h/pos/neg rows (384 descriptors
        each) -> X = [H|P|N] (128 x 3072) f32.  Replaces the baseline's
        36 gathers (SWDGE fixed cost ~1us each was the bottleneck).
      * DVE: D_p = H-P, D_n = H-N (f32 -> bf16)
      * PE: transpose D tiles (bf16, accum pairs in one PSUM group)
      * ACT: PSUM->SBUF pair copies;  DVE: masked copies (dT * mask)
      * PE: per tile, 3-matmul PSUM group computes [v_pos | v_neg]
      * ACT: Square -> sq;  DVE: sqd = sq_neg - sq_pos
      * PE: transpose sqd tiles; DVE: reduce -> per-row score diffs [128,8]
      * softplus tail on [128,8]; reg terms via Square/ttr accumulators
      * final: reduce + ones-matmul -> one f32 per core; host sums / M.
  - PE warmup transposes run during the gather latency so real matmuls hit
    the ramped p-state.
"""

import os
from contextlib import ExitStack

import numpy as np

import concourse.bass as bass
import concourse.tile as tile
from concourse import bacc, mybir
from concourse.masks import make_identity

M = 8192
E = 128
N_ENT = 500000
N_REL = 64
LAM = 1e-5
P = 128
N_CORES = 8
RPC = M // N_CORES          # rows per core = 1024
NT = RPC // P               # tiles per core = 8
NG = 4                      # gather groups (2 tiles/role per group)
TPG = NT // NG              # tiles per group = 2
N_WARM = 34                 # PE warmup transposes

f32 = mybir.dt.float32
bf16 = mybir.dt.bfloat16
i32 = mybir.dt.int32

_cache = {}

# ---- i32 blob column layout ----
_B_IDX = 0            # [128, 3*NT] i32 gather indices (grouped, see _plan)
_B_MCOL = _B_IDX + 3 * NT     # [128, NT] f32 suffix masks
_B_CNT = _B_MCOL + NT         # [128, 1] f32 cnt (rows 0:64 used)
_B_RFULL = _B_CNT + 1         # [128, E] f32 relation_embed (rows 0:64)
_B_COLS = _B_RFULL + E

# ---- bf16 rsm blob layout: [2, 3*NT*128] ----
#   cols [0 : NT*128)          rr   (row0 = r_A, row1 = r_D per tile)
#   cols [NT*128 : 3*NT*128)   rrhs (row0 = ones, row1 = [mrow|mrow] per tile)
_R_RR = 0
_R_RRHS = NT * P


def _build():
    nc = bacc.Bacc(
        "TRN2",
        target_bir_lowering=False,
        debug=False,
        num_devices=N_CORES,
    )

    ent = nc.dram_tensor("ent", (N_ENT, E), f32, kind="ExternalInput").ap()
    blob = nc.dram_tensor("blob", (P, _B_COLS), i32, kind="ExternalInput").ap()
    wab = nc.dram_tensor("wab", (P, 2 * NT * P), bf16, kind="ExternalInput").ap()
    rsm = nc.dram_tensor("rsm", (2, 3 * NT * P), bf16, kind="ExternalInput").ap()
    out = nc.dram_tensor("out", (1, 1), f32, kind="ExternalOutput").ap()

    with tile.TileContext(nc) as tc, ExitStack() as ctx:
        const = ctx.enter_context(tc.tile_pool(name="const", bufs=1))
        ps_pair = ctx.enter_context(tc.tile_pool(name="ps_pair", bufs=3, space="PSUM"))
        ps_vt = ctx.enter_context(tc.tile_pool(name="ps_vt", bufs=2, space="PSUM"))
        ps_sc = ctx.enter_context(tc.tile_pool(name="ps_sc", bufs=2, space="PSUM"))
        ps_fin = ctx.enter_context(tc.tile_pool(name="ps_fin", bufs=1, space="PSUM"))

        # ---- small inputs ----
        blob_sb = const.tile([P, _B_COLS], i32)
        nc.sync.dma_start(out=blob_sb[:], in_=blob[:])
        wab_sb = const.tile([P, 2 * NT * P], bf16)
        nc.sync.dma_start(out=wab_sb[:], in_=wab[:])
        rsm_sb = const.tile([2, 3 * NT * P], bf16)
        nc.sync.dma_start(out=rsm_sb[:], in_=rsm[:])

        idx_sb = blob_sb[:, _B_IDX : _B_IDX + 3 * NT]
        mcol_sb = blob_sb[:, _B_MCOL : _B_MCOL + NT].bitcast(f32)
        cnt_sb = blob_sb[:, _B_CNT : _B_CNT + 1].bitcast(f32)
        rfull_sb = blob_sb[:, _B_RFULL : _B_RFULL + E].bitcast(f32)

        # ---- constants ----
        iden = const.tile([P, P], f32)
        make_identity(nc, iden[:])
        iden_bf = const.tile([P, P], bf16)
        nc.scalar.copy(iden_bf[:], iden[:])
        ones_col = const.tile([P, 1], f32)
        nc.gpsimd.memset(ones_col[:], 1.0)
        acc = const.tile([P, 16], f32)
        nc.vector.memset(acc[:], 0.0)

        # ---- PE warmup (p-state ramp) ----
        wu_src = const.tile([P, P], bf16)
        nc.gpsimd.memset(wu_src[:], 0.0)
        for _ in range(N_WARM):
            wu_ps = ps_pair.tile([P, P], bf16, tag="pair")
            nc.tensor.matmul(
                out=wu_ps[:], lhsT=wu_src[:], rhs=iden_bf[:],
                is_transpose=True, start=True, stop=True,
            )

        # ---- gathers: X = [H | P | N] (each [128, RPC] f32) ----
        X = const.tile([P, 3 * RPC], f32)
        X3 = X[:].rearrange("p (r c) -> p r c", r=3)
        for g in range(NG):
            c0 = g * TPG * P
            nc.gpsimd.indirect_dma_start(
                out=X3[:, :, c0 : c0 + TPG * P],
                out_offset=None,
                in_=ent[:],
                in_offset=bass.IndirectOffsetOnAxis(
                    ap=idx_sb[:, 3 * TPG * g : 3 * TPG * (g + 1)], axis=0
                ),
            )

        xh = X[:, 0:RPC]
        xp = X[:, RPC : 2 * RPC]
        xn = X[:, 2 * RPC : 3 * RPC]

        # ---- working tiles ----
        dp = const.tile([P, RPC], bf16)
        dn = const.tile([P, RPC], bf16)
        dT = const.tile([P, 2, RPC], bf16)    # [pos | neg] transposed
        dTm = const.tile([P, 2, RPC], bf16)   # masked
        sq = const.tile([P, 2, RPC], bf16)
        sqd = const.tile([P, RPC], bf16)
        dsc = const.tile([P, NT], f32)
        xsq_a = const.tile([P, TPG * 3 * P], f32)   # ACT square scratch
        xsq_b = const.tile([P, TPG * 3 * P], f32)   # DVE ttr scratch
        xsq_c = const.tile([P, TPG * 3 * P], f32)   # Pool ttr scratch

        for g in range(NG):
            c0 = g * TPG * P
            cs = slice(c0, c0 + TPG * P)
            # reg partial for this group's gathered slab [128, 3, 256]
            gslab = X3[:, :, cs]
            if g in (0, 3):
                nc.scalar.activation(
                    out=xsq_a[:], in_=gslab,
                    func=mybir.ActivationFunctionType.Square,
                    accum_out=acc[:, 8 + g : 9 + g],
                )
            else:
                nc.vector.tensor_tensor_reduce(
                    out=xsq_b[:] if g == 1 else xsq_c[:], in0=gslab, in1=gslab,
                    scale=1.0, scalar=0.0,
                    op0=mybir.AluOpType.mult, op1=mybir.AluOpType.add,
                    accum_out=acc[:, 8 + g : 9 + g],
                )

            # D_p / D_n for this group's two tiles (f32 -> bf16)
            nc.vector.tensor_tensor(
                out=dp[:, cs], in0=xh[:, cs], in1=xp[:, cs],
                op=mybir.AluOpType.subtract,
            )
            nc.vector.tensor_tensor(
                out=dn[:, cs], in0=xh[:, cs], in1=xn[:, cs],
                op=mybir.AluOpType.subtract,
            )

            for j in range(g * TPG, (g + 1) * TPG):
                js = slice(j * P, (j + 1) * P)
                # transpose pair -> one PSUM group [128, 2, 128] bf16
                pair = ps_pair.tile([P, 2, P], bf16, tag="pair")
                nc.tensor.matmul(
                    out=pair[:, 0, :], lhsT=dp[:, js], rhs=iden_bf[:],
                    is_transpose=True, start=True, stop=False,
                )
                nc.tensor.matmul(
                    out=pair[:, 1, :], lhsT=dn[:, js], rhs=iden_bf[:],
                    is_transpose=True, start=False, stop=True,
                )
                # PSUM -> SBUF copy (ACT) and masked copy (DVE)
                nc.scalar.copy(dT[:, :, js], pair[:])
                nc.vector.tensor_scalar_mul(
                    out=dTm[:, :, js], in0=dT[:, :, js],
                    scalar1=mcol_sb[:, j : j + 1],
                )

                # vT = W_A^T dT + W_D^T dTm + [r_A; r_D] x [ones; mrow]
                vt = ps_vt.tile([P, 2 * P], f32, tag="vt")
                nc.tensor.matmul(
                    out=vt[:], lhsT=wab_sb[:, js], rhs=dT[:, :, js],
                    start=True, stop=False,
                )
                nc.tensor.matmul(
                    out=vt[:],
                    lhsT=wab_sb[:, (NT + j) * P : (NT + j + 1) * P],
                    rhs=dTm[:, :, js],
                    start=False, stop=False,
                )
                nc.tensor.matmul(
                    out=vt[:],
                    lhsT=rsm_sb[0:2, _R_RR + j * P : _R_RR + (j + 1) * P],
                    rhs=rsm_sb[0:2, _R_RRHS + j * 2 * P : _R_RRHS + (j + 1) * 2 * P],
                    start=False, stop=True,
                )

                # sq[:, :, js] = vt^2   (bf16 out)
                nc.scalar.activation(
                    out=sq[:, :, js], in_=vt[:],
                    func=mybir.ActivationFunctionType.Square,
                )

            # sqd = sq_neg - sq_pos for this group's columns
            nc.vector.tensor_tensor(
                out=sqd[:, cs], in0=sq[:, 1, cs], in1=sq[:, 0, cs],
                op=mybir.AluOpType.subtract,
            )

            # score transposes + partition reduce -> dsc[:, 2g:2g+2]
            scp = ps_sc.tile([P, 2, P], bf16, tag="scp")
            nc.tensor.matmul(
                out=scp[:, 0, :], lhsT=sqd[:, c0 : c0 + P], rhs=iden_bf[:],
                is_transpose=True, start=True, stop=False,
            )
            nc.tensor.matmul(
                out=scp[:, 1, :], lhsT=sqd[:, c0 + P : c0 + 2 * P], rhs=iden_bf[:],
                is_transpose=True, start=False, stop=True,
            )
            nc.vector.tensor_reduce(
                out=dsc[:, 2 * g : 2 * g + 2], in_=scp[:],
                axis=mybir.AxisListType.X, op=mybir.AluOpType.add,
            )

        # ---- softplus tail: softplus(0.5*dsc) -> acc[:, 0:8] ----
        t_abs = const.tile([P, NT], f32)
        nc.scalar.activation(
            out=t_abs[:], in_=dsc[:], func=mybir.ActivationFunctionType.Abs,
            scale=0.5,
        )
        t_exp = const.tile([P, NT], f32)
        nc.scalar.activation(
            out=t_exp[:], in_=t_abs[:], func=mybir.ActivationFunctionType.Exp,
            scale=-1.0,
        )
        t_ln = const.tile([P, NT], f32)
        nc.scalar.activation(
            out=t_ln[:], in_=t_exp[:], func=mybir.ActivationFunctionType.Ln,
            bias=1.0,
        )
        t_relu = const.tile([P, NT], f32)
        nc.scalar.activation(
            out=t_relu[:], in_=dsc[:], func=mybir.ActivationFunctionType.Relu,
            scale=0.5,
        )
        nc.vector.tensor_tensor(
            out=acc[:, 0:NT], in0=t_ln[:], in1=t_relu[:], op=mybir.AluOpType.add
        )

        # ---- reg terms ----
        # entity reg: acc[:, 8:12] *= 0.5*LAM
        nc.vector.tensor_scalar_mul(
            out=acc[:, 8:12], in0=acc[:, 8:12], scalar1=0.5 * LAM
        )
        # relation reg: cnt_k * 0.5*LAM * ||r_k||^2  (cnt pre-scaled on host)
        rsq_scr = const.tile([64, E], f32)
        rsq_col = const.tile([64, 1], f32)
        nc.vector.tensor_tensor_reduce(
            out=rsq_scr[:], in0=rfull_sb[0:64, :], in1=rfull_sb[0:64, :],
            scale=1.0, scalar=0.0,
            op0=mybir.AluOpType.mult, op1=mybir.AluOpType.add,
            accum_out=rsq_col[:],
        )
        nc.vector.tensor_tensor(
            out=acc[0:64, 12:13], in0=rsq_col[:], in1=cnt_sb[0:64, :],
            op=mybir.AluOpType.mult,
        )

        # ---- final scalar ----
        t_col = const.tile([P, 1], f32)
        nc.vector.tensor_reduce(
            out=t_col[:], in_=acc[:], axis=mybir.AxisListType.X,
            op=mybir.AluOpType.add,
        )
        fin_ps = ps_fin.tile([1, 1], f32)
        nc.tensor.matmul(
            out=fin_ps[:], lhsT=t_col[:], rhs=ones_col[:], start=True, stop=True
        )
        fin_sb = const.tile([1, 1], f32)
        nc.scalar.copy(fin_sb[:], fin_ps[:])
        nc.sync.dma_start(out=out[:], in_=fin_sb[:])

    nc.compile()
    return nc


def _choose_order(counts):
    """Permute relations so no 128-row window holds 2 boundaries."""
    rng = np.random.RandomState(0)
    for attempt in range(200):
        remaining = set(range(N_REL))
        orderp = []
        p = 0
        ok = True
        while remaining:
            cands = []
            for k in remaining:
                c = int(counts[k])
                viol = (
                    p > 0
                    and p + c < M
                    and (p // P) == ((p + c) // P)
                )
                if not viol:
                    cands.append((((p + c) % P), k))
            if not cands:
                ok = False
                break
            if attempt == 0:
                cands.sort()
                k = cands[-1][1]
            else:
                k = cands[rng.randint(len(cands))][1]
            orderp.append(k)
            p += int(counts[k])
            remaining.discard(k)
        if ok:
            return orderp
    raise RuntimeError("could not find a 1-boundary-per-tile relation order")


def _plan(h, r, pos_t, neg_t, relation_weight, relation_embed):
    counts = np.bincount(r, minlength=N_REL)
    perm = _choose_order(counts)
    order = np.concatenate(
        [np.flatnonzero(r == k) for k in perm if counts[k] > 0]
    ).astype(np.int64)
    assert order.shape[0] == M
    h_s = h[order]
    p_s = pos_t[order]
    n_s = neg_t[order]
    r_s = r[order]

    rw = relation_weight.astype(np.float32)
    re = relation_embed.astype(np.float32)

    maps = []
    for c in range(N_CORES):
        rows = slice(c * RPC, (c + 1) * RPC)
        hh = h_s[rows].reshape(NT, P).T.astype(np.int32)   # [128, NT]
        pp = p_s[rows].reshape(NT, P).T.astype(np.int32)
        nn = n_s[rows].reshape(NT, P).T.astype(np.int32)
        rc = r_s[rows]

        # grouped idx layout: per group g (TPG tiles/role):
        #   [H_{2g}, H_{2g+1}, P_{2g}, P_{2g+1}, N_{2g}, N_{2g+1}]
        idx = np.zeros((P, 3 * NT), np.int32)
        for g in range(NG):
            ts = slice(g * TPG, (g + 1) * TPG)
            base = 3 * TPG * g
            idx[:, base : base + TPG] = hh[:, ts]
            idx[:, base + TPG : base + 2 * TPG] = pp[:, ts]
            idx[:, base + 2 * TPG : base + 3 * TPG] = nn[:, ts]

        wab = np.zeros((P, 2 * NT * P), np.float32)
        rr = np.zeros((2, NT * P), np.float32)
        rrhs = np.zeros((2, NT * 2 * P), np.float32)
        mcol = np.zeros((P, NT), np.float32)
        for j in range(NT):
            tr = rc[j * P : (j + 1) * P]
            rel_a = int(tr[0])
            chg = np.flatnonzero(tr[1:] != tr[:-1])
            assert len(chg) <= 1, "tile with >1 relation boundary"
            wab[:, j * P : (j + 1) * P] = rw[rel_a]
            rr[0, j * P : (j + 1) * P] = re[rel_a]
            rrhs[0, j * 2 * P : (j + 1) * 2 * P] = 1.0
            if len(chg) == 1:
                b = int(chg[0]) + 1
                rel_b = int(tr[b])
                wab[:, (NT + j) * P : (NT + j + 1) * P] = rw[rel_b] - rw[rel_a]
                rr[1, j * P : (j + 1) * P] = re[rel_b] - re[rel_a]
                mrow = np.zeros(P, np.float32)
                mrow[b:] = 1.0
                rrhs[1, j * 2 * P : j * 2 * P + P] = mrow
                rrhs[1, j * 2 * P + P : (j + 1) * 2 * P] = mrow
                mcol[b:, j] = 1.0

        cnt = np.zeros((P, 1), np.float32)
        core_counts = np.bincount(rc, minlength=N_REL)
        cnt[:N_REL, 0] = core_counts * (0.5 * LAM)
        rfull = np.zeros((P, E), np.float32)
        rfull[:N_REL] = re

        blob = np.zeros((P, _B_COLS), np.int32)
        blob[:, _B_IDX : _B_IDX + 3 * NT] = idx
        blob[:, _B_MCOL : _B_MCOL + NT] = mcol.view(np.int32)
        blob[:, _B_CNT : _B_CNT + 1] = cnt.view(np.int32)
        blob[:, _B_RFULL : _B_RFULL + E] = rfull.view(np.int32)

        import ml_dtypes

        rsm = np.zeros((2, 3 * NT * P), np.float32)
        rsm[:, _R_RR : _R_RR + NT * P] = rr
        rsm[:, _R_RRHS : _R_RRHS + 2 * NT * P] = rrhs

        maps.append(
            {
                "blob": blob,
                "wab": wab.astype(ml_dtypes.bfloat16),
                "rsm": rsm.astype(ml_dtypes.bfloat16),
            }
        )
    return maps


def kernel(h, r, pos_t, neg_t, entity_embed, relation_embed, relation_weight):
    h = np.asarray(h).astype(np.int64)
    r = np.asarray(r).astype(np.int64)
    pos_t = np.asarray(pos_t).astype(np.int64)
    neg_t = np.asarray(neg_t).astype(np.int64)
    ent = np.ascontiguousarray(np.asarray(entity_embed, dtype=np.float32))
    re = np.ascontiguousarray(np.asarray(relation_embed, dtype=np.float32))
    rw = np.ascontiguousarray(np.asarray(relation_weight, dtype=np.float32))

    maps = _plan(h, r, pos_t, neg_t, rw, re)
    if "nc" not in _cache:
        _cache["nc"] = _build()
    nc = _cache["nc"]

    in_maps = [{"ent": ent, **maps[c]} for c in range(N_CORES)]

    if os.environ.get("KGE_SIM"):
        from concourse.bass_interp import CoreSim

        total = 0.0
        for c in range(N_CORES):
            sim = CoreSim(nc, trace=False)
            for name, arr in in_maps[c].items():
                sim.tensor(name)[:] = arr
            sim.simulate()
            total += float(sim.tensor("out")[0, 0])
        return np.float32(total / M)

    from concourse.bass_utils import run_bass_kernel_spmd

    res = run_bass_kernel_spmd(nc, in_maps, core_ids=list(range(N_CORES)))
    total = sum(float(res.results[c]["out"][0, 0]) for c in range(N_CORES))
    return np.float32(total / M)
